# revision 2
# baseline (speedup 1.0000x reference)
"""Mixture-of-nonlinear-experts policy net on 8 Trainium2 NeuronCores.

Sharding: data-parallel over batch B=32768 -> 4096 rows/core; all (small)
expert + gating weights replicated.  Each core computes its full slice of
(u, p, U) with no cross-core communication.  Device-side U is produced in
[E, Bshard, DU] layout (contiguous DMA writes straight out of PSUM->SBUF
tiles); the host gather transposes to the reference's [B, DU, E].

All matmuls run as float32r (FP22 truncate, 1 cycle/row on the PE for
free-dim >= 256, vs 4x for true fp32).  tanh/exp run on ScalarE with the
layer bias fused as the per-partition activation bias; the expert-weighted
sum u = sum_e p_e * U_e is a single fused multiply-add on VectorE per
(expert, 128-row block).
"""
import sys
if '/opt/trn_rl_repo' not in sys.path:
    sys.path.insert(0, '/opt/trn_rl_repo')
import numpy as np

B, DIN, DU, E, HE, HG = 32768, 512, 256, 16, 384, 264
NCORES = 8
BS = B // NCORES       # 4096 rows per core
NB = 256               # batch chunk (matmul free dim)
CHUNKS = BS // NB      # 16
SUBS = NB // 128       # 2

TRACE = False
last_results = None
_cache = {}


def _build():
    import concourse.bacc as bacc
    import concourse.mybir as mybir
    import concourse.tile as tile
    F32, F32R = mybir.dt.float32, mybir.dt.float32r
    Act = mybir.ActivationFunctionType
    Alu = mybir.AluOpType
    AX = mybir.AxisListType

    nc = bacc.Bacc("TRN2", target_bir_lowering=False, debug=False,
                   num_devices=NCORES)
    inpT_d = nc.dram_tensor("inpT", [DIN, BS], F32R, kind="ExternalInput").ap()
    w1_d = nc.dram_tensor("w1", [E, 4, 128, HE], F32R, kind="ExternalInput").ap()
    w2_d = nc.dram_tensor("w2", [E, 3, 128, DU], F32R, kind="ExternalInput").ap()
    gw1_d = nc.dram_tensor("gw1", [4, 128, HG], F32R, kind="ExternalInput").ap()
    gw2_d = nc.dram_tensor("gw2", [HG, E], F32R, kind="ExternalInput").ap()
    b1_d = nc.dram_tensor("b1", [128, E, 3], F32, kind="ExternalInput").ap()
    gb1_d = nc.dram_tensor("gb1", [128, 3], F32, kind="ExternalInput").ap()
    gb2_d = nc.dram_tensor("gb2", [128, E], F32, kind="ExternalInput").ap()
    b2_d = nc.dram_tensor("b2", [E, 128, DU], F32, kind="ExternalInput").ap()
    u_d = nc.dram_tensor("u", [BS, DU], F32, kind="ExternalOutput").ap()
    p_d = nc.dram_tensor("p", [BS, E], F32, kind="ExternalOutput").ap()
    U_d = nc.dram_tensor("U", [E, BS, DU], F32, kind="ExternalOutput").ap()

    GM = [(0, 128), (128, 256), (256, 264)]  # gating hidden m-slices

    with tile.TileContext(nc) as tc:
        with tc.tile_pool(name="wgt", bufs=1) as wgt, \
             tc.tile_pool(name="io", bufs=2) as io, \
             tc.tile_pool(name="hsb", bufs=3) as hsb, \
             tc.tile_pool(name="usb", bufs=4) as usb, \
             tc.tile_pool(name="acc", bufs=2) as accp, \
             tc.tile_pool(name="sm", bufs=4) as smp, \
             tc.tile_pool(name="hps", bufs=2, space="PSUM") as hps, \
             tc.tile_pool(name="ups", bufs=3, space="PSUM") as ups, \
             tc.tile_pool(name="lps", bufs=1, space="PSUM") as lps:

            # ---- resident weights ----
            w1_t, w2_t = [], []
            for e in range(E):
                t1 = wgt.tile([128, 4, HE], F32R, tag=f"w1_{e}")
                nc.sync.dma_start(t1[:], w1_d[e].rearrange("k p m -> p k m"))
                w1_t.append(t1)
                t2 = wgt.tile([128, 3, DU], F32R, tag=f"w2_{e}")
                nc.sync.dma_start(t2[:], w2_d[e].rearrange("k p m -> p k m"))
                w2_t.append(t2)
            gw1_t = wgt.tile([128, 4, HG], F32R, tag="gw1")
            nc.sync.dma_start(gw1_t[:], gw1_d.rearrange("k p m -> p k m"))
            gw2_t = wgt.tile([128, 3, E], F32R, tag="gw2")
            nc.sync.dma_start(gw2_t[:, 0:2, :],
                              gw2_d[0:256].rearrange("(k p) n -> p k n", p=128))
            nc.sync.dma_start(gw2_t[0:8, 2, :], gw2_d[256:264])
            b1_t = wgt.tile([128, E, 3], F32, tag="b1")
            nc.sync.dma_start(b1_t[:], b1_d)
            gb1_t = wgt.tile([128, 3], F32, tag="gb1")
            nc.sync.dma_start(gb1_t[:], gb1_d)
            gb2_t = wgt.tile([128, E], F32, tag="gb2")
            nc.sync.dma_start(gb2_t[:], gb2_d)
            b2_t = wgt.tile([128, E, DU], F32, tag="b2")
            nc.sync.dma_start(b2_t[:], b2_d.rearrange("e p m -> p e m"))

            inpT_r = inpT_d.rearrange("(k p) b -> p k b", p=128)

            for c in range(CHUNKS):
                col0 = c * NB
                x_t = io.tile([128, 4, NB], F32R, tag="x")
                nc.sync.dma_start(x_t[:], inpT_r[:, :, col0:col0 + NB])

                # ---- gating layer 1: g1 = tanh(gw1.T @ inp + gb1) ----
                g1_ps = hps.tile([128, 3, NB], F32, tag="hps")
                for mi, (m0, m1) in enumerate(GM):
                    mp = m1 - m0
                    for k in range(4):
                        nc.tensor.matmul(g1_ps[0:mp, mi, :],
                                         gw1_t[:, k, m0:m1], x_t[:, k, :],
                                         start=(k == 0), stop=(k == 3))
                g1_sb = hsb.tile([128, 3, NB], F32R, tag="h")
                for mi, (m0, m1) in enumerate(GM):
                    mp = m1 - m0
                    nc.scalar.activation(g1_sb[0:mp, mi, :], g1_ps[0:mp, mi, :],
                                         Act.Tanh, bias=gb1_t[0:mp, mi:mi + 1])

                # ---- gating layer 2 + softmax, per 128-row block ----
                p_ts = []
                for s in range(SUBS):
                    r0 = s * 128
                    lg_ps = lps.tile([128, E], F32, tag="lg")
                    for ki in range(3):
                        kp = GM[ki][1] - GM[ki][0]
                        nc.tensor.matmul(lg_ps[:],
                                         g1_sb[0:kp, ki, r0:r0 + 128],
                                         gw2_t[0:kp, ki, :],
                                         start=(ki == 0), stop=(ki == 2))
                    lg_sb = smp.tile([128, E], F32, tag="lg_sb")
                    nc.vector.tensor_tensor(lg_sb[:], lg_ps[:], gb2_t[:], op=Alu.add)
                    nmx = smp.tile([128, 1], F32, tag="nmx")
                    nc.vector.tensor_reduce(nmx[:], lg_sb[:], axis=AX.X,
                                            op=Alu.max, negate=True)
                    ex = smp.tile([128, E], F32, tag="ex")
                    ssum = smp.tile([128, 1], F32, tag="ssum")
                    nc.scalar.activation(ex[:], lg_sb[:], Act.Exp, bias=nmx[:],
                                         accum_out=ssum[:])
                    rin = smp.tile([128, 1], F32, tag="rin")
                    nc.vector.reciprocal(rin[:], ssum[:])
                    p_t = smp.tile([128, E], F32, tag="p")
                    nc.vector.tensor_scalar_mul(p_t[:], ex[:], rin[:])
                    nc.sync.dma_start(p_d[col0 + r0:col0 + r0 + 128, :], p_t[:])
                    p_ts.append(p_t)

                # ---- experts ----
                u_accs = [accp.tile([128, DU], F32, tag=f"uacc{s}",
                                    name=f"uacc{s}")
                          for s in range(SUBS)]
                for e in range(E):
                    h_ps = hps.tile([128, 3, NB], F32, tag="hps")
                    for m in range(3):
                        for k in range(4):
                            nc.tensor.matmul(h_ps[:, m, :],
                                             w1_t[e][:, k, m * 128:(m + 1) * 128],
                                             x_t[:, k, :],
                                             start=(k == 0), stop=(k == 3))
                    h_sb = hsb.tile([128, 3, NB], F32R, tag="h")
                    for m in range(3):
                        nc.scalar.activation(h_sb[:, m, :], h_ps[:, m, :],
                                             Act.Tanh, bias=b1_t[:, e, m:m + 1])
                    for s in range(SUBS):
                        r0 = s * 128
                        u_ps = ups.tile([128, DU], F32, tag="ups")
                        for k in range(3):
                            nc.tensor.matmul(u_ps[:],
                                             h_sb[:, k, r0:r0 + 128],
                                             w2_t[e][:, k, :],
                                             start=(k == 0), stop=(k == 2))
                        U_sb = usb.tile([128, DU], F32, tag="U")
                        nc.vector.tensor_tensor(U_sb[:], u_ps[:], b2_t[:, e, :],
                                                op=Alu.add)
                        nc.sync.dma_start(U_d[e, col0 + r0:col0 + r0 + 128, :],
                                          U_sb[:])
                        if e == 0:
                            nc.vector.tensor_scalar_mul(u_accs[s][:], U_sb[:],
                                                        p_ts[s][:, 0:1])
                        else:
                            nc.vector.scalar_tensor_tensor(
                                u_accs[s][:], U_sb[:], p_ts[s][:, e:e + 1],
                                u_accs[s][:], op0=Alu.mult, op1=Alu.add)
                for s in range(SUBS):
                    nc.sync.dma_start(u_d[col0 + s * 128:col0 + (s + 1) * 128, :],
                                      u_accs[s][:])
    nc.compile()
    return nc


def _get_nc():
    if 'nc' not in _cache:
        _cache['nc'] = _build()
    return _cache['nc']


def kernel(t, x, gw1, gb1, gw2, gb2, W1, b1, W2, b2):
    global last_results
    from concourse.bass_utils import run_bass_kernel_spmd

    nc = _get_nc()

    f32 = np.float32
    t = np.asarray(t, f32); x = np.asarray(x, f32)
    inpT = np.ascontiguousarray(np.concatenate([t, x], axis=1).T)  # [512, B]
    w1h = np.ascontiguousarray(np.asarray(W1, f32).reshape(E, 4, 128, HE))
    w2h = np.ascontiguousarray(np.asarray(W2, f32).reshape(E, 3, 128, DU))
    gw1h = np.ascontiguousarray(np.asarray(gw1, f32).reshape(4, 128, HG))
    gw2h = np.ascontiguousarray(np.asarray(gw2, f32))
    b1h = np.ascontiguousarray(
        np.asarray(b1, f32).reshape(E, 3, 128).transpose(2, 0, 1))   # [128,E,3]
    gb1p = np.zeros((3, 128), f32)
    gb1p.reshape(-1)[:HG] = np.asarray(gb1, f32)
    gb1h = np.ascontiguousarray(gb1p.T)                              # [128,3]
    gb2h = np.ascontiguousarray(
        np.broadcast_to(np.asarray(gb2, f32), (128, E)))
    b2h = np.ascontiguousarray(
        np.broadcast_to(np.asarray(b2, f32)[:, None, :], (E, 128, DU)))

    in_maps = []
    for c in range(NCORES):
        in_maps.append({
            "inpT": np.ascontiguousarray(inpT[:, c * BS:(c + 1) * BS]),
            "w1": w1h, "w2": w2h, "gw1": gw1h, "gw2": gw2h,
            "b1": b1h, "gb1": gb1h, "gb2": gb2h, "b2": b2h,
        })

    res = run_bass_kernel_spmd(nc, in_maps, core_ids=list(range(NCORES)),
                               trace=TRACE)
    last_results = res

    u = np.concatenate([r["u"] for r in res.results], axis=0)
    p = np.concatenate([r["p"] for r in res.results], axis=0)
    Ue = np.concatenate([r["U"] for r in res.results], axis=1)  # [E, B, DU]
    U = np.ascontiguousarray(Ue.transpose(1, 2, 0))             # [B, DU, E]
    return u, p, U
